# revision 6
# baseline (speedup 1.0000x reference)
"""GaussianKernel (KAN-style RBF layer) Trainium2 Bass kernel, V2.

reference:
    h = (grid_max - grid_min) / (num_grids - 1)        # 4/7
    basis = exp(-((x[..., None] - grid) / h) ** 2)     # [B, IN, G]
    out = basis.reshape(B, IN * G) @ spline_weight     # [B, OUT]

Shapes: x [16384, 512] f32, grid [8] f32, spline_weight [4096, 512] f32.

Data-parallel over 8 cores (2048 rows each). Per core, V2 design:

* Matmul cadence on TRN2 is ~216-259ns per 512-free-dim matmul (clock
  dependent) regardless of dtype; DoubleRow fp8 covers 2 contraction
  chunks per matmul. Time = 16 tiles x (32 - n_fp8_chunks/2) x cadence.
  The fp8 chunk set is chosen by exact host-side error simulation
  against the 2e-2 gate.

* Basis production avoids the 8-DERF ACT bottleneck (~80us) via the exact
  factorization f_g(x) = f_0(x) * E(x)^g * const_g with E = exp(2*d*x/h^2):
  ACT computes Square/Exp only (one table set, no ACT table reloads):
    u0 = Square(1.75x+3.5); p0 = Exp(-u0) = f_0
    E1 = Exp(3.5x); E2 = Exp(7x); E4 = Exp(14x)
    u3 = Square(1.75x+0.5); f3 = Exp(-u3)   (grid 3 direct, highest energy)
  DVE chains p_g = p_{g'} * E_k (6 tensor_tensors, binary decomposition
  bounds table-error accumulation at <=3 factors).

* fp8 grids are MEAN-CENTERED before quantization ((p*s - mu) -> fp8 via
  DVE tensor_scalar affine casts) which cuts fp8 error energy ~1.6x; the
  exact bias B[o] = sum mu_g * w8_g[i,o] is restored by the PSUM->SBUF
  copy (DVE tensor_tensor add with a broadcast bias tile).

* Weight-side fp8 rounding error is reduced host-side (free): regression
  compensation of the predictable part through the bf16 chunks.

* bc0 is processed per ic-half with each half's matmuls covering all 4
  psum tiles (2-stage accumulation) so the tensor engine lights up after
  ~1/2 of bc0's ACT work instead of all of it.
"""

import os
from contextlib import ExitStack

import numpy as np

import concourse.bass as bass
import concourse.bacc as bacc
import concourse.mybir as mybir
import concourse.tile as tile

N_CORES = 8
BATCH = 16384
B_CORE = BATCH // N_CORES  # 2048
IN_F = 512
OUT_F = 512
G = 8
B_CHUNK = 512
N_BC = B_CORE // B_CHUNK   # 4
N_IC = 4                   # 128-row in-feature blocks

FP32 = mybir.dt.float32
BF16 = mybir.dt.bfloat16
F8 = mybir.dt.float8e4
SQ = mybir.ActivationFunctionType.Square
EXP = mybir.ActivationFunctionType.Exp
COPY = mybir.ActivationFunctionType.Copy
ALU = mybir.AluOpType
DR = mybir.MatmulPerfMode.DoubleRow

GRID_MIN, GRID_MAX = -2.0, 2.0
H = (GRID_MAX - GRID_MIN) / (G - 1)
AEXP = 2.0 * H / H**2  # 2*delta/h^2 = 3.5
GRIDS = np.linspace(GRID_MIN, GRID_MAX, G)

# fp8 (grid, ic-half) set; half 0 = in-features 0..255, 1 = 256..511.
GK_CFG = os.environ.get("GK_CFG", "A")
_CFGS = {
    "A": [(g, h) for g in (0, 1, 6, 7) for h in (0, 1)],
    "B": [(g, h) for g in (0, 1, 6, 7) for h in (0, 1)] + [(2, 0)],
    "B2": [(g, h) for g in (0, 1, 6, 7) for h in (0, 1)] + [(2, 0), (5, 1)],
    "C": [(g, h) for g in (0, 1, 2, 6, 7) for h in (0, 1)],
}
FP8H = _CFGS[GK_CFG]
FP8_GRIDS = sorted({g for g, _ in FP8H})
N_DRP = len(FP8H)
BF_CHUNKS = [(g, ic) for g in range(G) for ic in range(N_IC)
             if (g, ic // 2) not in FP8H]
N_BFC = len(BF_CHUNKS)
N_MM = N_BFC + N_DRP  # matmuls per 128-row tile

OUT_BF16 = os.environ.get("GK_OUT_BF16", "0") == "1"
OUT_DT = BF16 if OUT_BF16 else FP32
W_OPT = os.environ.get("GK_W_OPT", "1") == "1"
# fp8 affine casts on ACT for the first GK_ACT_CASTS grids (rest on DVE)
ACT_CASTS = int(os.environ.get("GK_ACT_CASTS", "0"))

# chain bookkeeping: p_g = f_g * RG[g], RG[g] = exp((g_g^2 - g_0^2)/h^2)
RG = np.exp((GRIDS**2 - GRIDS[0]**2) / H**2)
# analytic means under N(0,1)
MUS = (H / np.sqrt(H**2 + 2.0)) * np.exp(-GRIDS**2 / (H**2 + 2.0))


def gaussian_kernel(ctx: ExitStack, tc: tile.TileContext,
                    out_ap: bass.AP, xt_ap: bass.AP,
                    wb_ap, w8_ap, bias_ap):
    nc = tc.nc

    const_pool = ctx.enter_context(tc.tile_pool(name="const", bufs=1))
    w_pool = ctx.enter_context(tc.tile_pool(name="w", bufs=1))
    x_pool = ctx.enter_context(tc.tile_pool(name="x", bufs=2))
    u_pool = ctx.enter_context(tc.tile_pool(name="u", bufs=2))
    p_pool = ctx.enter_context(tc.tile_pool(name="p", bufs=2))
    q_pool = ctx.enter_context(tc.tile_pool(name="q", bufs=2))
    out_pool = ctx.enter_context(tc.tile_pool(name="out_stage", bufs=6))
    psum_pool = ctx.enter_context(
        tc.tile_pool(name="psum_acc", bufs=8, space="PSUM"))

    b35 = const_pool.tile([128, 1], FP32, tag="b35")
    nc.gpsimd.memset(b35[:], 3.5)
    b05 = const_pool.tile([128, 1], FP32, tag="b05")
    nc.gpsimd.memset(b05[:], 0.5)
    bz = const_pool.tile([128, 1], FP32, tag="bz")
    nc.gpsimd.memset(bz[:], 0.0)

    # warm the exp_and_others table (square/exp/copy all live there)
    warm = const_pool.tile([128, 1], BF16, tag="warm")
    nc.scalar.activation(warm[:], bz[:], EXP, bias=bz[:], scale=1.0)

    # bias tile [128, 512] fp32 (replicated rows) for the output stage
    bias_sb = const_pool.tile([128, OUT_F], FP32, tag="bias")
    nc.scalar.dma_start(bias_sb[:], bias_ap)

    # ---- weights: resident in SBUF, streamed on the ACT HWDGE queue ----
    w8_sb = None
    if N_DRP:
        w8_sb = w_pool.tile([128, N_DRP, 2, OUT_F], F8, tag="w8")
        w8_src = w8_ap.rearrange("p (d t o) -> p d t o", d=N_DRP, t=2, o=OUT_F)
        c8 = min(2, N_DRP)
        nc.scalar.dma_start(w8_sb[:, 0:c8], w8_src[:, 0:c8])
        if c8 < N_DRP:
            nc.scalar.dma_start(w8_sb[:, c8:], w8_src[:, c8:])
    wb_sb = w_pool.tile([128, N_BFC, OUT_F], BF16, tag="wb")
    wb_src = wb_ap.rearrange("p (c o) -> p c o", c=N_BFC, o=OUT_F)
    nc.scalar.dma_start(wb_sb[:, 0:4, :], wb_src[:, 0:4, :])
    wb_mid = N_BFC // 2
    nc.sync.dma_start(wb_sb[:, 4:wb_mid, :], wb_src[:, 4:wb_mid, :])
    nc.scalar.dma_start(wb_sb[:, wb_mid:, :], wb_src[:, wb_mid:, :])

    xt_src = xt_ap.rearrange("(nb p) (ic b) -> nb p ic b",
                             nb=N_BC, p=128, ic=N_IC, b=B_CHUNK)

    fp8set = set(FP8H)

    def prep_half(tiles, half):
        """ACT/DVE ops producing basis for ic-half `half` (ic 2h..2h+1)."""
        x_t, u_t, p, f3, E1, E2, E4, q = tiles
        sl = slice(2 * half, 2 * half + 2)
        # direct grid 3 first (earliest-ready bf16 chunks for the head)
        nc.scalar.activation(u_t[:, 1, sl], x_t[:, sl], SQ,
                             bias=b05[:], scale=1.75)
        nc.scalar.activation(f3[:, sl], u_t[:, 1, sl], EXP,
                             bias=bz[:], scale=-1.0)
        nc.scalar.activation(u_t[:, 0, sl], x_t[:, sl], SQ,
                             bias=b35[:], scale=1.75)
        nc.scalar.activation(p[0][:, sl], u_t[:, 0, sl], EXP,
                             bias=bz[:], scale=-1.0)
        nc.scalar.activation(E4[:, sl], x_t[:, sl], EXP,
                             bias=bz[:], scale=4.0 * AEXP)
        nc.scalar.activation(E1[:, sl], x_t[:, sl], EXP,
                             bias=bz[:], scale=1.0 * AEXP)
        nc.scalar.activation(E2[:, sl], x_t[:, sl], EXP,
                             bias=bz[:], scale=2.0 * AEXP)
        # DVE chain
        nc.vector.tensor_tensor(p[4][:, sl], p[0][:, sl], E4[:, sl],
                                op=ALU.mult)
        nc.vector.tensor_tensor(p[1][:, sl], p[0][:, sl], E1[:, sl],
                                op=ALU.mult)
        nc.vector.tensor_tensor(p[2][:, sl], p[0][:, sl], E2[:, sl],
                                op=ALU.mult)
        nc.vector.tensor_tensor(p[5][:, sl], p[4][:, sl], E1[:, sl],
                                op=ALU.mult)
        nc.vector.tensor_tensor(p[6][:, sl], p[4][:, sl], E2[:, sl],
                                op=ALU.mult)
        nc.vector.tensor_tensor(p[7][:, sl], p[6][:, sl], E1[:, sl],
                                op=ALU.mult)
        # fp8 affine casts for this half: q_g = (p_g/RG[g] - mu_g) -> fp8
        for j, g in enumerate(FP8_GRIDS):
            if (g, half) not in fp8set:
                continue
            s = float(1.0 / RG[g])
            m = float(MUS[g])
            if j < ACT_CASTS:
                nc.scalar.activation(q[g][:, sl], p[g][:, sl], COPY,
                                     bias=-m, scale=s)
            else:
                nc.vector.tensor_scalar(q[g][:, sl], p[g][:, sl],
                                        s, m,
                                        op0=ALU.mult, op1=ALU.subtract)

    def alloc_tiles(bc, split_dma=False):
        x_t = x_pool.tile([128, N_IC, B_CHUNK], BF16, tag="xt")
        if split_dma:
            nc.sync.dma_start(x_t[:, 0:2, :], xt_src[bc, :, 0:2, :])
            nc.sync.dma_start(x_t[:, 2:4, :], xt_src[bc, :, 2:4, :])
        else:
            nc.sync.dma_start(x_t[:], xt_src[bc])
        u_t = u_pool.tile([128, 2, N_IC, B_CHUNK], FP32, tag="u")
        p = {g: p_pool.tile([128, N_IC, B_CHUNK], BF16, tag=f"p{g}",
                            name=f"p{g}")
             for g in (0, 1, 2, 4, 5, 6, 7)}
        f3 = p_pool.tile([128, N_IC, B_CHUNK], BF16, tag="f3")
        E1 = p_pool.tile([128, N_IC, B_CHUNK], BF16, tag="E1")
        E2 = p_pool.tile([128, N_IC, B_CHUNK], BF16, tag="E2")
        E4 = p_pool.tile([128, N_IC, B_CHUNK], BF16, tag="E4")
        q = {g: q_pool.tile([128, N_IC, B_CHUNK], F8, tag=f"q{g}",
                            name=f"q{g}")
             for g in FP8_GRIDS}
        return (x_t, u_t, p, f3, E1, E2, E4, q)

    def mm_half(tiles, half, paccs, first, last):
        """Matmuls touching ic-half `half`, for all 4 tiles of the bc."""
        _, _, p, f3, _, _, _, q = tiles
        drs = [(d, g, h) for d, (g, h) in enumerate(FP8H) if h == half]
        bfs = [(c, g, ic) for c, (g, ic) in enumerate(BF_CHUNKS)
               if ic // 2 == half]
        for bt in range(4):
            bsl = slice(bt * 128, (bt + 1) * 128)
            ops = ([("bf", *o) for o in bfs] + [("dr", *o) for o in drs])
            n = len(ops)
            for k, op in enumerate(ops):
                st = first and (k == 0)
                sp = last and (k == n - 1)
                if op[0] == "dr":
                    _, d, g, h = op
                    nc.tensor.matmul(
                        paccs[bt][:], q[g][:, 2 * h:2 * h + 2, bsl],
                        w8_sb[:, d], start=st, stop=sp, perf_mode=DR)
                else:
                    _, c, g, ic = op
                    src = f3 if g == 3 else p[g]
                    nc.tensor.matmul(
                        paccs[bt][:], src[:, ic:ic + 1, bsl],
                        wb_sb[:, c:c + 1, :], start=st, stop=sp)

    def flush_out(bc, paccs):
        for bt in range(4):
            os_t = out_pool.tile([128, OUT_F], OUT_DT, tag="os")
            nc.vector.tensor_tensor(os_t[:], paccs[bt][:], bias_sb[:],
                                    op=ALU.add)
            nc.sync.dma_start(
                out_ap[bc * B_CHUNK + bt * 128:
                       bc * B_CHUNK + (bt + 1) * 128, :],
                os_t[:])

    def alloc_paccs(bc):
        return [psum_pool.tile([128, OUT_F], FP32, tag="pacc",
                               name=f"pc{bc}_{t}")
                for t in range(4)]

    # ---- schedule ----
    # bc0: per-half pipeline; each half's matmuls cover all 4 psum tiles.
    tiles0 = alloc_tiles(0, split_dma=True)
    prep_half(tiles0, 0)
    paccs0 = alloc_paccs(0)
    mm_half(tiles0, 0, paccs0, first=True, last=False)
    prep_half(tiles0, 1)
    tiles1 = alloc_tiles(1)
    prep_half(tiles1, 0)
    mm_half(tiles0, 1, paccs0, first=False, last=True)
    flush_out(0, paccs0)
    prep_half(tiles1, 1)

    tiles_cur = tiles1
    for bc in range(1, N_BC):
        tiles_next = None
        if bc + 1 < N_BC:
            tiles_next = alloc_tiles(bc + 1)
            prep_half(tiles_next, 0)
            prep_half(tiles_next, 1)
        paccs = alloc_paccs(bc)
        mm_half(tiles_cur, 0, paccs, first=True, last=False)
        mm_half(tiles_cur, 1, paccs, first=False, last=True)
        flush_out(bc, paccs)
        tiles_cur = tiles_next


_CACHE = {}


def _build():
    key = (GK_CFG, OUT_BF16, ACT_CASTS)
    if key in _CACHE:
        return _CACHE[key]
    nc = bacc.Bacc("TRN2", target_bir_lowering=False, debug=False,
                   num_devices=N_CORES)
    xt_t = nc.dram_tensor("xt", [N_BC * 128, N_IC * B_CHUNK], BF16,
                          kind="ExternalInput")
    wb_t = nc.dram_tensor("wb", [128, N_BFC * OUT_F], BF16,
                          kind="ExternalInput")
    w8_t = (nc.dram_tensor("w8", [128, N_DRP * 2 * OUT_F], F8,
                           kind="ExternalInput") if N_DRP else None)
    bias_t = nc.dram_tensor("bias", [128, OUT_F], FP32, kind="ExternalInput")
    out_t = nc.dram_tensor("out", [B_CORE, OUT_F], OUT_DT,
                           kind="ExternalOutput")
    with tile.TileContext(nc) as tc:
        with ExitStack() as ctx:
            gaussian_kernel(ctx, tc, out_t.ap(), xt_t.ap(), wb_t.ap(),
                            w8_t.ap() if w8_t is not None else None,
                            bias_t.ap())
    nc.compile()
    _CACHE[key] = nc
    return nc


def _prep_weights(spline_weight: np.ndarray):
    """Host-side weight packing: bf16 chunks (with p-fold), fp8 DR pairs
    (+ regression compensation), bias row."""
    import ml_dtypes

    w3 = np.ascontiguousarray(spline_weight, dtype=np.float64).reshape(
        IN_F, G, OUT_F)

    w8_blocks = {}   # (g, half) -> [256, OUT] dequantized fp8 values
    for g, half in FP8H:
        blk = w3[half * 256:(half + 1) * 256, g, :]
        w8_blocks[(g, half)] = blk.astype(np.float32).astype(
            ml_dtypes.float8_e4m3).astype(np.float64)

    wb_adj = {}      # (g, ic) -> [128, OUT] float64 adjustable copy
    for g, ic in BF_CHUNKS:
        wb_adj[(g, ic)] = w3[ic * 128:(ic + 1) * 128, g, :].copy()

    bias = np.zeros(OUT_F, dtype=np.float64)

    if W_OPT and FP8H:
        # Regression compensation: the part of the fp8 weight rounding
        # error delta_w predictable from bf16-grid basis values is folded
        # into the bf16 weights (and its mean part into the bias).
        xs, wq = np.polynomial.hermite_e.hermegauss(201)
        dens = wq / wq.sum()
        F = np.exp(-(((xs[:, None] - GRIDS) / H) ** 2))
        mu = dens @ F
        C = (F - mu).T @ (dens[:, None] * (F - mu))  # centered covariance
        bf_grids = sorted({g for g, _ in BF_CHUNKS})
        Cnn = C[np.ix_(bf_grids, bf_grids)]
        for g, half in FP8H:
            dw = w8_blocks[(g, half)] - w3[half * 256:(half + 1) * 256, g, :]
            cgn = C[g, bf_grids]
            A = np.linalg.solve(Cnn + 1e-12 * np.eye(len(bf_grids)), cgn)
            for n, a in zip(bf_grids, A):
                if abs(a) < 1e-4:
                    continue
                for ic in (2 * half, 2 * half + 1):
                    if (n, ic) in wb_adj:
                        rows = slice((ic - 2 * half) * 128,
                                     (ic - 2 * half + 1) * 128)
                        wb_adj[(n, ic)] -= a * dw[rows, :]
                        # keep the mean contribution unchanged:
                        # d(out) = f_n * (-a dw) = (v_n + mu_n)(-a dw)
                        bias += MUS[n] * (a * dw[rows, :]).sum(axis=0)

    wb_list = []
    for g, ic in BF_CHUNKS:
        wblk = wb_adj[(g, ic)]
        if g != 3:
            wblk = wblk / RG[g]  # p_g = f_g * RG[g]  =>  w~ = w / RG[g]
        wb_list.append(wblk.astype(np.float32))
    wb = np.stack(wb_list, axis=0)  # [N_BFC, 128, OUT]
    wb = np.ascontiguousarray(
        wb.transpose(1, 0, 2).reshape(128, N_BFC * OUT_F)
    ).astype(ml_dtypes.bfloat16)

    w8 = None
    if FP8H:
        blocks = []
        for g, half in FP8H:
            blk = w8_blocks[(g, half)]  # [256, OUT]
            bias += MUS[g] * blk.sum(axis=0)
            blocks.append(blk.reshape(2, 128, OUT_F))
        w8s = np.stack(blocks, axis=0)  # [N_DRP, 2, 128, OUT]
        w8 = np.ascontiguousarray(
            w8s.transpose(2, 0, 1, 3).reshape(128, N_DRP * 2 * OUT_F)
        ).astype(ml_dtypes.float8_e4m3)

    bias_full = np.ascontiguousarray(
        np.broadcast_to(bias.astype(np.float32), (128, OUT_F)))
    return wb, w8, bias_full


def kernel(x: np.ndarray, grid: np.ndarray, spline_weight: np.ndarray,
           _want_results=False, **_kw) -> np.ndarray:
    from concourse.bass_utils import run_bass_kernel_spmd
    import ml_dtypes

    nc = _build()
    wb, w8, bias_full = _prep_weights(spline_weight)

    # x pre-transposed per core (pure layout prep): [core, bc, p128, ic, b]
    x = np.ascontiguousarray(x, dtype=np.float32)
    xt = np.ascontiguousarray(
        x.reshape(N_CORES, N_BC, B_CHUNK, N_IC, 128)
        .transpose(0, 1, 4, 3, 2)
        .reshape(N_CORES, N_BC * 128, N_IC * B_CHUNK)).astype(
            ml_dtypes.bfloat16)

    in_maps = []
    for i in range(N_CORES):
        m = {"xt": xt[i], "wb": wb, "bias": bias_full}
        if w8 is not None:
            m["w8"] = w8
        in_maps.append(m)
    res = run_bass_kernel_spmd(nc, in_maps, list(range(N_CORES)))
    out = np.concatenate(
        [np.asarray(res.results[i]["out"], dtype=np.float32)
         for i in range(N_CORES)], axis=0)
    if _want_results:
        return out, res
    return out
